# revision 14
# baseline (speedup 1.0000x reference)
"""GRU (B=64, T=512, DIN=D=512) on 8 Trainium2 NeuronCores.

Strategy
--------
Data-parallel over batch: each core owns BL = 8 batch rows, weights are
replicated (per the sharding hint).  Per core:

1. Projection phase: xg = X @ W_g + b_g for g in {z, r, h} as GEMMs with W
   stationary and X^T streaming, written straight into an SBUF-resident
   pre-activation buffer xall[p, g, m, t*BL+b] (bf16, ~96KB/partition) by
   ScalarE Identity-with-bias ops.  Projection chunks 0-1 run as a
   prologue; the remaining chunks are interleaved into the scan's PE idle
   windows (chunk c+2 is emitted during scan chunk c), so projection time
   is almost entirely hidden.

2. Scan phase (the sequential part): state is kept transposed,
   hT [128 partitions = d%128, KT=4 k-tiles, BL=8], so that
   - the recurrent matmuls are psum[m] += U[k,m].T @ hmT[k] (U stationary,
     state streaming, output already transposed), and
   - all elementwise work (sigmoid/tanh/blend) runs on fat [128, 32] tiles.
   The x-projection term is accumulated into PSUM by an identity matmul
   (start=True) so the activations read PSUM directly - no DVE pre-adds.
   The update gate is computed as zc = sigmoid(-zpre) = 1 - z (free affine
   scale=-1 on the ACT op), which turns the blend into
       h = (hm - zc*hm) + zc*hh
   where (hm - zc*hm) is computed off the critical path; only zc*hh and
   the final add sit between tanh and the next step's matmuls, and those
   run in k-halves so the next step's k0/k1 matmuls start after half the
   blend.

The mask input: reference semantics are h_t = z*(m_{t-1}*h_{t-1}) + ...,
i.e. the *shifted* mask multiplies the previous state.  For the all-ones
mask (what setup_inputs produces) this is the identity, so the fast path
skips the multiply; a general path (host-broadcast shifted mask streamed
from DRAM, one extra DVE mul per step) handles arbitrary 0/1 masks.
"""

import numpy as np
from contextlib import ExitStack

import concourse.bass as bass
import concourse.bacc as bacc
import concourse.mybir as mybir
import concourse.tile as tile
from concourse.bass_utils import run_bass_kernel_spmd

FP32 = mybir.dt.float32
BF16 = mybir.dt.bfloat16
AF = mybir.ActivationFunctionType

B, T, DIN, D = 64, 512, 512, 512
NCORES = 8
BL = B // NCORES            # 8 batch rows per core
KT = DIN // 128             # 4 contraction tiles
MT = D // 128               # 4 output tiles
P = 128


def build_nc(T_=T, masked=False, use_bf16=True):
    """Build the single-core SPMD program (identical on all 8 cores)."""
    tl = min(64, T_)                     # steps per chunk
    sch = T_ // tl                       # chunks
    pcw = tl * BL                        # chunk width in columns (512)
    ldt = BF16 if use_bf16 else FP32     # low-precision dtype

    nc = bacc.Bacc(None, target_bir_lowering=False, debug=False)

    xT = nc.dram_tensor("xT", [DIN, T_ * BL], FP32, kind="ExternalInput")
    w_lay = {g: nc.dram_tensor(f"W{g}", [P, KT * D], FP32, kind="ExternalInput")
             for g in "zrh"}
    u_lay = {g: nc.dram_tensor(f"U{g}", [P, KT * D], FP32, kind="ExternalInput")
             for g in "zrh"}
    b4 = {g: nc.dram_tensor(f"b{g}", [P, MT], FP32, kind="ExternalInput")
          for g in "zrh"}
    eye_d = nc.dram_tensor("eye", [P, P], FP32, kind="ExternalInput")
    mb = None
    if masked:
        mb = nc.dram_tensor("mb", [T_, P, KT * BL], FP32, kind="ExternalInput")
    hT_out = nc.dram_tensor("hT_out", [D, BL], FP32, kind="ExternalOutput")

    with tile.TileContext(nc) as tc, ExitStack() as ctx:
        upool = ctx.enter_context(tc.tile_pool(name="upool", bufs=1))
        wpool = ctx.enter_context(tc.tile_pool(name="wpool", bufs=1))
        bp = ctx.enter_context(tc.tile_pool(name="bp", bufs=1))
        xap = ctx.enter_context(tc.tile_pool(name="xap", bufs=1))
        xtp = ctx.enter_context(tc.tile_pool(name="xtp", bufs=2 * KT))
        pproj = ctx.enter_context(
            tc.tile_pool(name="pproj", bufs=2, space="PSUM"))
        psc = ctx.enter_context(tc.tile_pool(name="psc", bufs=2, space="PSUM"))
        sm = ctx.enter_context(tc.tile_pool(name="sm", bufs=3))
        mbp = ctx.enter_context(tc.tile_pool(name="mbp", bufs=2))

        u_sb = {}
        eye_sb = upool.tile([P, P], ldt, tag="eye", name="eye")
        if use_bf16:
            eye_stage = upool.tile([P, P], FP32, tag="eyestage", name="eyestage")
            nc.sync.dma_start(eye_stage[:], eye_d[:])
            nc.vector.tensor_copy(eye_sb[:], eye_stage[:])
        else:
            nc.sync.dma_start(eye_sb[:], eye_d[:])
        w_sb = {}
        b_sb = {}
        for g in "zrh":
            if use_bf16:
                stage = upool.tile([P, KT * D], FP32, tag="ustage", name="ustage")
                nc.sync.dma_start(stage[:], u_lay[g][:])
                u_sb[g] = upool.tile([P, KT * D], BF16, tag=f"u{g}", name=f"u{g}")
                nc.vector.tensor_copy(u_sb[g][:], stage[:])
            else:
                u_sb[g] = upool.tile([P, KT * D], FP32, tag=f"u{g}", name=f"u{g}")
                nc.sync.dma_start(u_sb[g][:], u_lay[g][:])
            w_sb[g] = wpool.tile([P, KT * D], FP32, tag=f"w{g}", name=f"w{g}")
            nc.sync.dma_start(w_sb[g][:], w_lay[g][:])
            b_sb[g] = bp.tile([P, MT], FP32, tag=f"b{g}", name=f"b{g}")
            nc.sync.dma_start(b_sb[g][:], b4[g][:])

        # SBUF-resident pre-activations: [p, gate, m-tile, t*BL+b]
        xall = xap.tile([P, 3, KT, T_ * BL], ldt, tag="xall", name="xall")

        gate_i = {"z": 0, "r": 1, "h": 2}
        xt_tiles = {}

        def emit_xt_dmas(c):
            tiles = []
            for kk in range(KT):
                xt = xtp.tile([P, pcw], FP32, tag="xt", name=f"xt{c}_{kk}")
                nc.sync.dma_start(
                    xt[:], xT[kk * P:(kk + 1) * P, c * pcw:(c + 1) * pcw])
                tiles.append(xt)
            xt_tiles[c] = tiles

        def emit_proj_unit(c, g, m):
            ps = pproj.tile([P, pcw], FP32, tag="pp", name=f"pp{c}{g}{m}")
            for kk in range(KT):
                nc.tensor.matmul(
                    ps[:],
                    w_sb[g][:, kk * D + m * P: kk * D + (m + 1) * P],
                    xt_tiles[c][kk][:],
                    start=(kk == 0), stop=(kk == KT - 1))
            nc.scalar.activation(
                xall[:, gate_i[g], m, c * pcw:(c + 1) * pcw], ps[:],
                AF.Identity, bias=b_sb[g][:, m:m + 1])

        proj_units = [(c, g, m) for c in range(sch)
                      for g in "zrh" for m in range(MT)]
        # prologue: chunks 0-2 run dense (also warms up the PE's HAM clock
        # gate before the scan starts); chunk c+2 interleaves into scan
        # chunk c for the rest
        n_pro = min(sch, 3)
        for c in range(n_pro):
            emit_xt_dmas(c)
        for c, g, m in [u for u in proj_units if u[0] < n_pro]:
            emit_proj_unit(c, g, m)
        rest = [u for u in proj_units if u[0] >= n_pro]

        # MM emission order: k-halves outer, because the previous step's
        # blend produces the state in k-halves (h0 then h1) - all k0/k1
        # matmuls can start as soon as the first half lands.
        ORD_K = ([(kk, m) for kk in (0, 1) for m in range(MT)]
                 + [(kk, m) for kk in (2, 3) for m in range(MT)])

        def gate_mms(psum, g, rhs, xv, order):
            # identity matmul accumulates the x-projection into PSUM first
            # (start=True, one MM covers all 4 m-regions); it has no data
            # deps beyond the projection, so PE can issue it while waiting
            # for rhs.
            nc.tensor.matmul(psum[:], eye_sb[:], xv[:],
                             start=True, stop=False)
            for i, (kk, m) in enumerate(order):
                nc.tensor.matmul(
                    psum[:, m],
                    u_sb[g][:, kk * D + m * P: kk * D + (m + 1) * P],
                    rhs[:, kk],
                    start=False,
                    stop=(i == len(order) - 1))

        h_prev = sm.tile([P, KT, BL], ldt, tag="h", name="h0")
        nc.vector.memset(h_prev[:], 0.0)

        for t in range(T_):
            c = t // tl
            ti = t % tl
            if ti == 0:
                if n_pro <= c + 2 < sch:
                    emit_xt_dmas(c + 2)
                if masked:
                    mb_sb = mbp.tile([P, tl, KT * BL], FP32, tag="m",
                                     name=f"mb{c}")
                    nc.sync.dma_start(
                        mb_sb[:],
                        mb[c * tl:(c + 1) * tl].rearrange("t p x -> p t x"))

            if masked:
                hm = sm.tile([P, KT, BL], ldt, tag="hm")
                nc.vector.tensor_mul(
                    hm[:], h_prev[:],
                    mb_sb[:, ti].rearrange("p (k b) -> p k b", k=KT))
            else:
                hm = h_prev

            xv = xall[:, :, :, t * BL:(t + 1) * BL]

            # r gate
            ps_r = psc.tile([P, KT, BL], FP32, tag="pr")
            gate_mms(ps_r, "r", hm, xv[:, 1], ORD_K)
            r_sb = sm.tile([P, KT, BL], ldt, tag="r")
            nc.scalar.activation(r_sb[:], ps_r[:], AF.Sigmoid)
            rhm = sm.tile([P, KT, BL], ldt, tag="rhm")
            nc.vector.tensor_mul(rhm[:], r_sb[:], hm[:])

            # z gate (complement): zc = 1 - z = sigmoid(-zpre)
            ps_z = psc.tile([P, KT, BL], FP32, tag="pz")
            gate_mms(ps_z, "z", hm, xv[:, 0], ORD_K)
            zc = sm.tile([P, KT, BL], ldt, tag="zc")
            nc.scalar.activation(zc[:], ps_z[:], AF.Sigmoid, scale=-1.0)
            # off-critical-path part of the blend: c1 = hm - zc*hm
            zchm = sm.tile([P, KT, BL], ldt, tag="zchm")
            nc.vector.tensor_mul(zchm[:], zc[:], hm[:])
            c1 = sm.tile([P, KT, BL], ldt, tag="c1")
            nc.vector.tensor_sub(c1[:], hm[:], zchm[:])

            # h candidate
            ps_h = psc.tile([P, KT, BL], FP32, tag="ph")
            gate_mms(ps_h, "h", rhm, xv[:, 2], ORD_K)

            # interleave one hidden projection unit into this step's tail
            # window (PE would otherwise idle here); chunk c+2 during scan
            # chunk c, starting at scan chunk n_pro-2 so early chunks run
            # a clean warm rhythm
            if (rest and n_pro - 2 <= c and c + 2 == rest[0][0]
                    and ti % 5 == 0 and (ti // 5) < 12):
                emit_proj_unit(*rest.pop(0))

            # critical tail in k-halves: h = c1 + zc*hh; the next step's
            # k0/k1 matmuls only need the first half of h.
            hh = sm.tile([P, KT, BL], ldt, tag="hh")
            b2 = sm.tile([P, KT, BL], ldt, tag="b2")
            h_new = sm.tile([P, KT, BL], ldt, tag="h")
            for hf in range(2):
                sl = slice(2 * hf, 2 * hf + 2)
                nc.scalar.activation(hh[:, sl], ps_h[:, sl], AF.Tanh)
                nc.vector.tensor_mul(b2[:, sl], zc[:, sl], hh[:, sl])
                nc.vector.tensor_add(h_new[:, sl], c1[:, sl], b2[:, sl])
            h_prev = h_new

        hout = sm.tile([P, KT, BL], FP32, tag="hout", name="hout")
        nc.vector.tensor_copy(hout[:], h_prev[:])
        for kk in range(KT):
            nc.sync.dma_start(hT_out[kk * P:(kk + 1) * P, :], hout[:, kk])

    nc.compile()
    return nc


_NC_CACHE = {}


def _get_nc(masked, use_bf16=True):
    key = (masked, use_bf16)
    if key not in _NC_CACHE:
        _NC_CACHE[key] = build_nc(T, masked=masked, use_bf16=use_bf16)
    return _NC_CACHE[key]


def _w_layout(w):
    # [DIN, D] -> [128, KT*D] with lay[p, kk*D + j] = w[kk*128 + p, j]
    return np.ascontiguousarray(
        w.reshape(KT, P, D).transpose(1, 0, 2).reshape(P, KT * D), dtype=np.float32)


def _b_layout(b):
    return np.ascontiguousarray(b.reshape(MT, P).T, dtype=np.float32)


def make_in_maps(X, W_z, U_z, b_z, W_r, U_r, b_r, W_h, U_h, b_h, mask,
                 masked):
    X = np.asarray(X, dtype=np.float32)
    shared = {"eye": np.eye(P, dtype=np.float32)}
    for g, w, u, b in (("z", W_z, U_z, b_z), ("r", W_r, U_r, b_r),
                       ("h", W_h, U_h, b_h)):
        shared[f"W{g}"] = _w_layout(np.asarray(w, dtype=np.float32))
        shared[f"U{g}"] = _w_layout(np.asarray(u, dtype=np.float32))
        shared[f"b{g}"] = _b_layout(np.asarray(b, dtype=np.float32))

    in_maps = []
    for c in range(NCORES):
        bsl = slice(c * BL, (c + 1) * BL)
        m = dict(shared)
        m["xT"] = np.ascontiguousarray(
            X[bsl].transpose(2, 1, 0).reshape(DIN, T * BL))
        if masked:
            msh = np.zeros((T, BL), dtype=np.float32)
            msh[1:] = np.asarray(mask)[bsl, :T - 1].T.astype(np.float32)
            m["mb"] = np.ascontiguousarray(
                np.tile(msh[:, None, :], (1, P, KT)))
        in_maps.append(m)
    return in_maps


def kernel(X, W_z, U_z, b_z, W_r, U_r, b_r, W_h, U_h, b_h, mask):
    mask = np.asarray(mask)
    masked = not bool(np.all(mask[:, :T - 1] == 1))
    nc = _get_nc(masked)
    in_maps = make_in_maps(X, W_z, U_z, b_z, W_r, U_r, b_r, W_h, U_h, b_h,
                           mask, masked)
    res = run_bass_kernel_spmd(nc, in_maps, core_ids=list(range(NCORES)))
    out = np.empty((B, D), dtype=np.float32)
    for c in range(NCORES):
        out[c * BL:(c + 1) * BL] = res.results[c]["hT_out"].T
    return out
